# revision 4
# baseline (speedup 1.0000x reference)
"""AxialSelfAttention2d on 8 TRN2 NeuronCores.

Sharding: phase 1 (row attention over L) is data-parallel over the 128 (b, s)
pairs: core c owns b = c//4 and a 16-row s-slab. Phase 2 (column attention
over S) is parallel over l: core c owns l-slice [c*32, (c+1)*32) of BOTH
batches. The reshard between phases is a single 8-way AllToAll.

Matmuls run in bf16 (fp32 PSUM accumulation); softmax and LayerNorm
statistics and the residual paths are fp32.
"""

import sys

sys.path.insert(0, "/opt/trn_rl_repo")

import numpy as np
import ml_dtypes

import concourse.bass as bass
import concourse.tile as tile
from concourse import bacc, mybir
from concourse.bass_utils import run_bass_kernel_spmd

B, S, L, D, H, C = 2, 64, 256, 512, 8, 64
E3 = 3 * D
NCORES = 8
S_LOC = S // 4          # 16 s rows per core (phase 1)
L_LOC = L // NCORES     # 32 l cols per core (phase 2)
EPS = 1e-5

F32 = mybir.dt.float32
BF16 = mybir.dt.bfloat16
AF = mybir.ActivationFunctionType
ALU = mybir.AluOpType


def _ln_block(nc, sb, eps_t, tres, out_tile, gb=None):
    """LayerNorm along the free dim of tres [128, 512] -> out_tile (fp32)."""
    stats = sb.tile([128, 6], F32, tag="ln_stats", name="ln_stats")
    nc.vector.bn_stats(stats, tres)
    mv = sb.tile([128, 2], F32, tag="ln_mv", name="ln_mv")
    nc.vector.bn_aggr(mv, stats)
    std = sb.tile([128, 1], F32, tag="ln_std", name="ln_std")
    nc.scalar.activation(std, mv[:, 1:2], func=AF.Sqrt, bias=eps_t[:, 0:1])
    rstd = sb.tile([128, 1], F32, tag="ln_rstd", name="ln_rstd")
    nc.vector.reciprocal(rstd, std)
    nmr = sb.tile([128, 1], F32, tag="ln_nmr", name="ln_nmr")
    nc.vector.tensor_mul(nmr, mv[:, 0:1], rstd)
    nc.vector.tensor_scalar_mul(nmr, nmr, -1.0)
    if gb is None:
        nc.scalar.activation(out_tile, tres, func=AF.Identity,
                             scale=rstd[:, 0:1], bias=nmr[:, 0:1])
    else:
        g_sb, b_sb = gb
        tmp = sb.tile([128, D], F32, tag="ln_tmp", name="ln_tmp")
        nc.scalar.activation(tmp, tres, func=AF.Identity,
                             scale=rstd[:, 0:1], bias=nmr[:, 0:1])
        nc.vector.tensor_mul(tmp, tmp, g_sb)
        nc.vector.tensor_add(out_tile, tmp, b_sb)


def build_program(apply_gb1: bool, apply_gb2: bool):
    nc = bacc.Bacc("TRN2", target_bir_lowering=False, debug=False,
                   num_devices=NCORES)

    # ---- per-core I/O ----
    x_nat = nc.dram_tensor("x_nat", [S_LOC, L, D], F32, kind="ExternalInput").ap()
    xt_in = nc.dram_tensor("xt_in", [4, 128, S_LOC, L], BF16, kind="ExternalInput").ap()
    w_row = nc.dram_tensor("w_rowT", [4, 128, E3], BF16, kind="ExternalInput").ap()
    w_col = nc.dram_tensor("w_colT", [4, 128, E3], BF16, kind="ExternalInput").ap()
    bqk_row = nc.dram_tensor("bqk_row", [128, 8], F32, kind="ExternalInput").ap()
    bqk_col = nc.dram_tensor("bqk_col", [128, 8], F32, kind="ExternalInput").ap()
    bv_col = nc.dram_tensor("bv_col", [128, D], F32, kind="ExternalInput").ap()
    gb_ins = {}
    for flag, names in ((apply_gb1, ("g1b", "beta1b")), (apply_gb2, ("g2b", "beta2b"))):
        if flag:
            for n in names:
                gb_ins[n] = nc.dram_tensor(n, [128, D], F32, kind="ExternalInput").ap()
    out = nc.dram_tensor("out", [B, S, L_LOC, D], F32, kind="ExternalOutput").ap()

    with tile.TileContext(nc) as tc:
        with tc.tile_pool(name="const", bufs=1) as const, \
             tc.tile_pool(name="dram", bufs=1, space="DRAM") as dram, \
             tc.tile_pool(name="sb", bufs=2) as sb, \
             tc.tile_pool(name="sbh", bufs=3) as sbh, \
             tc.tile_pool(name="p_qk", bufs=2, space="PSUM") as p_qk, \
             tc.tile_pool(name="p_v", bufs=2, space="PSUM") as p_v, \
             tc.tile_pool(name="p_st", bufs=2, space="PSUM") as p_st, \
             tc.tile_pool(name="p_av", bufs=2, space="PSUM") as p_av:

            # ---- constants ----
            w1 = const.tile([128, 4, E3], BF16, name="w1")
            nc.sync.dma_start(out=w1, in_=w_row.rearrange("t p e -> p t e"))
            w2 = const.tile([128, 4, E3], BF16, name="w2")
            nc.sync.dma_start(out=w2, in_=w_col.rearrange("t p e -> p t e"))
            bqk1 = const.tile([128, 8], F32, name="bqk1")
            nc.sync.dma_start(out=bqk1, in_=bqk_row)
            bqk2 = const.tile([128, 8], F32, name="bqk2")
            nc.sync.dma_start(out=bqk2, in_=bqk_col)
            bv2 = const.tile([128, D], F32, name="bv2")
            nc.sync.dma_start(out=bv2, in_=bv_col)
            eps_t = const.tile([128, 1], F32, name="eps_t")
            nc.vector.memset(eps_t, EPS)
            gb1 = gb2 = None
            if apply_gb1:
                g1s = const.tile([128, D], F32, name="g1s")
                nc.sync.dma_start(out=g1s, in_=gb_ins["g1b"])
                b1s = const.tile([128, D], F32, name="b1s")
                nc.sync.dma_start(out=b1s, in_=gb_ins["beta1b"])
                gb1 = (g1s, b1s)
            if apply_gb2:
                g2s = const.tile([128, D], F32, name="g2s")
                nc.sync.dma_start(out=g2s, in_=gb_ins["g2b"])
                b2s = const.tile([128, D], F32, name="b2s")
                nc.sync.dma_start(out=b2s, in_=gb_ins["beta2b"])
                gb2 = (g2s, b2s)

            # ---- AllToAll bounce buffers ----
            # a2a_in: [dest 8, s_loc 16, l_in 32, D]
            a2a_in = dram.tile([NCORES, S_LOC, L_LOC, D], F32, name="a2a_in")
            a2a_out = dram.tile([NCORES, S_LOC, L_LOC, D], F32, name="a2a_out")

            # ================= phase 1: row attention =================
            for s in range(S_LOC):
                xt_s = sb.tile([128, 4, L], BF16, tag="xt_s", name="xt_s")
                nc.sync.dma_start(out=xt_s, in_=xt_in[:, :, s, :].rearrange("t p l -> p t l"))
                x_s = sb.tile([128, 2, D], F32, tag="x_s", name="x_s")
                nc.sync.dma_start(out=x_s, in_=x_nat[s].rearrange("(lt p) d -> p lt d", p=128))

                # Q,K projection -> [e_tile 8, tok 256] bf16 (e = h*64+c)
                qk_sb = sb.tile([128, 8, L], BF16, tag="qk_sb", name="qk_sb")
                for et in range(8):
                    ps = p_qk.tile([128, L], F32, tag="ps_qk", name="ps_qk")
                    for dc in range(4):
                        nc.tensor.matmul(ps, w1[:, dc, et * 128:(et + 1) * 128],
                                         xt_s[:, dc, :],
                                         start=(dc == 0), stop=(dc == 3))
                    nc.scalar.activation(qk_sb[:, et, :], ps, func=AF.Identity,
                                         bias=bqk1[:, et:et + 1])
                # V projection -> natural [tok, (h, c)] + ones column
                vo_sb = sb.tile([128, 2, H, C + 1], BF16, tag="vo_sb", name="vo_sb")
                nc.vector.memset(vo_sb[:, :, :, C:C + 1], 1.0)
                for lt in range(2):
                    psv = p_v.tile([128, D], F32, tag="ps_v", name="ps_v")
                    for dc in range(4):
                        nc.tensor.matmul(psv, xt_s[:, dc, lt * 128:(lt + 1) * 128],
                                         w1[:, dc, 2 * D:3 * D],
                                         start=(dc == 0), stop=(dc == 3))
                    nc.vector.tensor_copy(out=vo_sb[:, lt, :, 0:C],
                                          in_=psv.rearrange("p (h c) -> p h c", h=H))

                ro_sb = sb.tile([128, 2, D], F32, tag="ro_sb", name="ro_sb")
                for h in range(H):
                    t, r = h // 2, (h % 2) * 64
                    et_sb = sbh.tile([128, 2, L], BF16, tag="et_sb", name="et_sb")
                    for jt in range(2):
                        ps_s = p_st.tile([128, L], F32, tag="ps_st", name="ps_st")
                        nc.tensor.matmul(ps_s[0:128, :],
                                         qk_sb[r:r + 64, 4 + t, jt * 128:(jt + 1) * 128],
                                         qk_sb[r:r + 64, t, :],
                                         start=True, stop=True)
                        nc.scalar.activation(et_sb[:, jt, :], ps_s, func=AF.Exp)
                    for it in range(2):
                        ps_a = p_av.tile([128, C + 1], F32, tag="ps_av", name="ps_av")
                        for jt in range(2):
                            nc.tensor.matmul(ps_a, et_sb[:, jt, it * 128:(it + 1) * 128],
                                             vo_sb[:, jt, h, :],
                                             start=(jt == 0), stop=(jt == 1))
                        rz = sbh.tile([128, 1], F32, tag="rz", name="rz")
                        nc.vector.reciprocal(rz, ps_a[:, C:C + 1])
                        nc.scalar.activation(ro_sb[:, it, h * C:(h + 1) * C],
                                             ps_a[:, 0:C], func=AF.Copy,
                                             scale=rz[:, 0:1])

                # residual + LN1, write into a2a_in
                for lt in range(2):
                    tres = sb.tile([128, D], F32, tag="tres", name="tres")
                    nc.vector.tensor_add(tres, x_s[:, lt, :], ro_sb[:, lt, :])
                    o1 = sb.tile([128, D], F32, tag="o1", name="o1")
                    _ln_block(nc, sb, eps_t, tres, o1, gb1)
                    dst = a2a_in[lt * 4:(lt + 1) * 4, s, :, :]
                    nc.sync.dma_start(out=dst, in_=o1)

            # ================= reshard =================
            nc.gpsimd.collective_compute(
                "AllToAll", ALU.bypass,
                replica_groups=[list(range(NCORES))],
                ins=[a2a_in.opt()], outs=[a2a_out.opt()])

            # a2a_out viewed [src 8, s_in 16, l 32, D]; src = b*4 + s//16
            # ================= phase 2: column attention =================
            for b in range(B):
                for ch in range(L_LOC // 4):      # 4 l per chunk, 256 tokens
                    o1n = sb.tile([128, 2, D], F32, tag="o1n", name="o1n")
                    xt2 = sb.tile([128, 4, 256], BF16, tag="xt2", name="xt2")
                    for tt in range(2):
                        l0 = ch * 4 + tt * 2
                        src = a2a_out[b * 4:(b + 1) * 4, :, l0:l0 + 2, :]
                        nc.sync.dma_start(
                            out=o1n[:, tt, :],
                            in_=src.rearrange("src si lc d -> lc src si d"))
                        o1b = sb.tile([128, D], BF16, tag="o1b", name="o1b")
                        nc.scalar.activation(o1b, o1n[:, tt, :], func=AF.Copy)
                        nc.sync.dma_start(out=xt2[:, :, tt * 128:(tt + 1) * 128],
                                          in_=o1b, transpose=True)

                    qk2_sb = sb.tile([128, 8, 256], BF16, tag="qk2_sb", name="qk2_sb")
                    for et in range(8):
                        ps2 = p_qk.tile([128, 256], F32, tag="ps_qk", name="ps2")
                        for dc in range(4):
                            nc.tensor.matmul(ps2, w2[:, dc, et * 128:(et + 1) * 128],
                                             xt2[:, dc, :],
                                             start=(dc == 0), stop=(dc == 3))
                        nc.scalar.activation(qk2_sb[:, et, :], ps2, func=AF.Identity,
                                             bias=bqk2[:, et:et + 1])
                    vo2_sb = sb.tile([128, 2, H, C + 1], BF16, tag="vo2_sb", name="vo2_sb")
                    nc.vector.memset(vo2_sb[:, :, :, C:C + 1], 1.0)
                    for tt in range(2):
                        psv2 = p_v.tile([128, D], F32, tag="ps_v", name="psv2")
                        for dc in range(4):
                            nc.tensor.matmul(psv2, xt2[:, dc, tt * 128:(tt + 1) * 128],
                                             w2[:, dc, 2 * D:3 * D],
                                             start=(dc == 0), stop=(dc == 3))
                        nc.vector.tensor_copy(out=vo2_sb[:, tt, :, 0:C],
                                              in_=psv2.rearrange("p (h c) -> p h c", h=H))

                    co_sb = sb.tile([128, 2, D], F32, tag="co_sb", name="co_sb")
                    for lp in range(2):           # l pairs: (0,1), (2,3)
                        for h in range(H):
                            t, r = h // 2, (h % 2) * 64
                            ps_s2 = p_st.tile([128, 64], F32, tag="ps_st", name="ps_s2")
                            for li in (2 * lp, 2 * lp + 1):
                                o = (li % 2) * 64
                                tok = li * 64
                                nc.tensor.matmul(
                                    ps_s2[o:o + 64, :],
                                    qk2_sb[r:r + 64, 4 + t, tok:tok + 64],
                                    qk2_sb[r:r + 64, t, tok:tok + 64],
                                    start=True, stop=True)
                            et2 = sbh.tile([128, 64], BF16, tag="et2", name="et2")
                            nc.scalar.activation(et2, ps_s2, func=AF.Exp)
                            ps_a2 = p_av.tile([128, C + 1], F32, tag="ps_av", name="ps_a2")
                            for li in (2 * lp, 2 * lp + 1):
                                o = (li % 2) * 64
                                nc.tensor.matmul(
                                    ps_a2[o:o + 64, :],
                                    et2[o:o + 64, :],
                                    vo2_sb[o:o + 64, li // 2, h, :],
                                    start=True, stop=True)
                            rz2 = sbh.tile([128, 1], F32, tag="rz", name="rz2")
                            nc.vector.reciprocal(rz2, ps_a2[:, C:C + 1])
                            nc.scalar.activation(co_sb[:, lp, h * C:(h + 1) * C],
                                                 ps_a2[:, 0:C], func=AF.Copy,
                                                 scale=rz2[:, 0:1])

                    for tt in range(2):
                        t2 = sb.tile([128, D], F32, tag="t2", name="t2")
                        nc.vector.tensor_add(t2, o1n[:, tt, :], co_sb[:, tt, :])
                        nc.vector.tensor_add(t2, t2, bv2)
                        o2 = sb.tile([128, D], F32, tag="o2", name="o2")
                        _ln_block(nc, sb, eps_t, t2, o2, gb2)
                        l0 = ch * 4 + tt * 2
                        dst = out[b, :, l0:l0 + 2, :].rearrange("s lc d -> lc s d")
                        nc.sync.dma_start(out=dst, in_=o2)

    nc.compile()
    return nc


_PROGRAM_CACHE = {}


def _get_program(apply_gb1, apply_gb2):
    key = (apply_gb1, apply_gb2)
    if key not in _PROGRAM_CACHE:
        _PROGRAM_CACHE[key] = build_program(apply_gb1, apply_gb2)
    return _PROGRAM_CACHE[key]


def _prep_in_maps(x, w_row, b_row, w_col, b_col, g1, beta1, g2, beta2,
                  apply_gb1, apply_gb2):
    bf16 = ml_dtypes.bfloat16
    b_row_v = b_row[2 * D:3 * D].astype(np.float32)
    # w^T tiles: w_T[t, p, e] = w[e, t*128+p]
    w1t = np.ascontiguousarray(
        w_row.T.reshape(4, 128, E3).astype(bf16))
    w2t = np.ascontiguousarray(
        w_col.T.reshape(4, 128, E3).astype(bf16))
    bqk1 = np.ascontiguousarray(b_row[:2 * D].reshape(8, 128).T).astype(np.float32)
    bqk2 = np.ascontiguousarray(b_col[:2 * D].reshape(8, 128).T).astype(np.float32)
    bv2 = np.broadcast_to(b_col[2 * D:3 * D], (128, D)).astype(np.float32)
    bv2 = np.ascontiguousarray(bv2)

    in_maps = []
    for c in range(NCORES):
        b = c // 4
        s0 = (c % 4) * S_LOC
        xs = np.asarray(x[b, s0:s0 + S_LOC])            # [16, 256, 512]
        x_nat = (xs + b_row_v[None, None, :]).astype(np.float32)
        # xt[t, p, s, l] = x[s, l, t*128+p]
        xt = np.ascontiguousarray(
            xs.transpose(2, 0, 1).reshape(4, 128, S_LOC, L)).astype(bf16)
        m = {
            "x_nat": np.ascontiguousarray(x_nat),
            "xt_in": xt,
            "w_rowT": w1t,
            "w_colT": w2t,
            "bqk_row": bqk1,
            "bqk_col": bqk2,
            "bv_col": bv2,
        }
        if apply_gb1:
            m["g1b"] = np.ascontiguousarray(
                np.broadcast_to(g1, (128, D)).astype(np.float32))
            m["beta1b"] = np.ascontiguousarray(
                np.broadcast_to(beta1, (128, D)).astype(np.float32))
        if apply_gb2:
            m["g2b"] = np.ascontiguousarray(
                np.broadcast_to(g2, (128, D)).astype(np.float32))
            m["beta2b"] = np.ascontiguousarray(
                np.broadcast_to(beta2, (128, D)).astype(np.float32))
        in_maps.append(m)
    return in_maps


def kernel(x, w_row, b_row, w_col, b_col, g1, beta1, g2, beta2):
    x = np.asarray(x, dtype=np.float32)
    w_row = np.asarray(w_row, dtype=np.float32)
    b_row = np.asarray(b_row, dtype=np.float32)
    w_col = np.asarray(w_col, dtype=np.float32)
    b_col = np.asarray(b_col, dtype=np.float32)
    g1 = np.asarray(g1, dtype=np.float32)
    beta1 = np.asarray(beta1, dtype=np.float32)
    g2 = np.asarray(g2, dtype=np.float32)
    beta2 = np.asarray(beta2, dtype=np.float32)

    apply_gb1 = not (np.all(g1 == 1.0) and np.all(beta1 == 0.0))
    apply_gb2 = not (np.all(g2 == 1.0) and np.all(beta2 == 0.0))

    nc = _get_program(apply_gb1, apply_gb2)
    in_maps = _prep_in_maps(x, w_row, b_row, w_col, b_col, g1, beta1, g2, beta2,
                            apply_gb1, apply_gb2)
    res = run_bass_kernel_spmd(nc, in_maps, core_ids=list(range(NCORES)))
    full = np.empty((B, S, L, D), dtype=np.float32)
    for c in range(NCORES):
        full[:, :, c * L_LOC:(c + 1) * L_LOC, :] = \
            res.results[c]["out"].reshape(B, S, L_LOC, D)
    return full
